# revision 1
# baseline (speedup 1.0000x reference)
"""Trainium2 Bass kernel for the LDE1D vq_codebook problem.

Math (per batch b):
    q[t,k]   = 2*s0 * x[t,:] @ mu[k,:]          (PE, bf16 in / fp32 accum)
    p[t,k]   = exp(q[t,k])                      (ACT)
    pu[t,k]  = p[t,k] * u[k],  u = exp(-s0*||mu_k||^2)   (DVE ttr)
    D[t]     = sum_k pu[t,k]                    (same DVE ttr, accum_out)
    w[t,k]   = pu[t,k] * weights[t] / D[t]      (DVE)
    acc[k,:] = sum_t w[t,k] * [x[t,:], 1]       (PE, accumulated in PSUM)
    e[k,d]   = acc[k,d] / acc[k,D] - mu[k,d]
Softmax shift-invariance: the -s0*||x||^2 term is constant over k and drops.
The per-k factor u[k] cancels in acc[k,d]/acc[k,D], so using pu instead of
the exact softmax numerator is algebraically exact.

Sharding: data-parallel over B across 8 cores (8 batches each), mu/s
replicated. exp args are bounded (~|2*x.mu| <= ~20) so no max-subtract.
"""

import sys
from contextlib import ExitStack

import numpy as np

sys.path.insert(0, "/opt/trn_rl_repo")

import ml_dtypes

import concourse.bass as bass
import concourse.tile as tile
from concourse import bacc, mybir
from concourse.bass_utils import run_bass_kernel_spmd

BF16 = mybir.dt.bfloat16
F32 = mybir.dt.float32

B, T, D, K = 64, 4096, 256, 64
NCORES = 8
BPC = B // NCORES  # batches per core
TT = 128           # tokens per tile (partition dim)


def build_program(bpc=BPC, t=T, trn_type="TRN2"):
    ntiles = t // TT
    nc = bacc.Bacc(trn_type, target_bir_lowering=False, debug=False,
                   num_devices=NCORES)
    x_d = nc.dram_tensor("x", [bpc, t, D], F32, kind="ExternalInput").ap()
    wsT_d = nc.dram_tensor("wsT", [bpc, TT, ntiles], F32,
                           kind="ExternalInput").ap()
    muT2_d = nc.dram_tensor("muT2", [128, 2 * K], BF16,
                            kind="ExternalInput").ap()
    urep_d = nc.dram_tensor("urep", [128, K], BF16, kind="ExternalInput").ap()
    mu_d = nc.dram_tensor("mu", [K, D], F32, kind="ExternalInput").ap()
    ident_d = nc.dram_tensor("ident", [128, 128], BF16,
                             kind="ExternalInput").ap()
    out_d = nc.dram_tensor("out", [bpc, K, D], F32, kind="ExternalOutput").ap()

    with tile.TileContext(nc) as tc, ExitStack() as ctx:
        _body(ctx, tc, out_d, x_d, wsT_d, muT2_d, urep_d, mu_d, ident_d,
              bpc, ntiles)
    nc.compile()
    return nc


def _body(ctx, tc, out_d, x_d, wsT_d, muT2_d, urep_d, mu_d, ident_d,
          bpc, ntiles):
    nc = tc.nc
    const = ctx.enter_context(tc.tile_pool(name="const", bufs=1))
    muT2 = const.tile([128, 2 * K], BF16)
    nc.sync.dma_start(muT2[:], muT2_d[:])
    urep = const.tile([128, K], BF16)
    nc.sync.dma_start(urep[:], urep_d[:])
    mu_sb = const.tile([K, D], F32)
    nc.sync.dma_start(mu_sb[:], mu_d[:])
    ident = const.tile([128, 128], BF16)
    nc.sync.dma_start(ident[:], ident_d[:])

    xin_pool = ctx.enter_context(tc.tile_pool(name="xin", bufs=4))
    xbf_pool = ctx.enter_context(tc.tile_pool(name="xbf", bufs=4))
    xt_pool = ctx.enter_context(tc.tile_pool(name="xt", bufs=3))
    p_pool = ctx.enter_context(tc.tile_pool(name="p", bufs=3))
    pu_pool = ctx.enter_context(tc.tile_pool(name="pu", bufs=3))
    w_pool = ctx.enter_context(tc.tile_pool(name="w", bufs=3))
    sc_pool = ctx.enter_context(tc.tile_pool(name="sc", bufs=4))
    ws_pool = ctx.enter_context(tc.tile_pool(name="ws", bufs=2))
    res_pool = ctx.enter_context(tc.tile_pool(name="res", bufs=2))
    pt_psum = ctx.enter_context(tc.tile_pool(name="pt", bufs=2, space="PSUM"))
    pq_psum = ctx.enter_context(tc.tile_pool(name="pq", bufs=2, space="PSUM"))
    pe_psum = ctx.enter_context(tc.tile_pool(name="pe", bufs=2, space="PSUM"))

    for b in range(bpc):
        ws = ws_pool.tile([TT, ntiles], F32)
        nc.sync.dma_start(ws[:], wsT_d[b])
        acc = pe_psum.tile([K, D + 1], F32)
        for ti in range(ntiles):
            # load + cast
            xin = xin_pool.tile([TT, D], F32)
            nc.sync.dma_start(xin[:], x_d[b, ti * TT:(ti + 1) * TT, :])
            xbf = xbf_pool.tile([TT, D + 1], BF16)
            nc.gpsimd.tensor_copy(xbf[:, 0:D], xin[:])
            nc.gpsimd.memset(xbf[:, D:D + 1], 1.0)
            # transpose x tile (two 128x128 halves) via PE
            pt = pt_psum.tile([128, D], BF16)
            nc.tensor.transpose(pt[:, 0:128], xbf[:, 0:128], ident[:])
            nc.tensor.transpose(pt[:, 128:256], xbf[:, 128:256], ident[:])
            xt = xt_pool.tile([128, D], BF16)
            nc.scalar.copy(xt[:], pt[:])
            # q = x @ (2 s0 mu)^T : contract d in two halves
            pq = pq_psum.tile([TT, K], F32)
            nc.tensor.matmul(pq[:], xt[:, 0:128], muT2[:, 0:K],
                             start=True, stop=False)
            nc.tensor.matmul(pq[:], xt[:, 128:256], muT2[:, K:2 * K],
                             start=False, stop=True)
            # p = exp(q)  (bf16 out)
            p = p_pool.tile([TT, K], BF16)
            nc.scalar.activation(p[:], pq[:], mybir.ActivationFunctionType.Exp)
            # pu = p * u ;  D_t = sum_k pu
            pu = pu_pool.tile([TT, K], BF16)
            nc.vector.tensor_mul(pu[:], p[:], urep[:])
            dt = sc_pool.tile([TT, 1], F32, tag="dt")
            nc.vector.reduce_sum(dt[:], pu[:], axis=mybir.AxisListType.X)
            # scale_t = weights_t / D_t ; w = pu * scale_t
            rd = sc_pool.tile([TT, 1], F32, tag="rd")
            nc.vector.reciprocal(rd[:], dt[:])
            scl = sc_pool.tile([TT, 1], F32, tag="scl")
            nc.vector.tensor_tensor(scl[:], ws[:, ti:ti + 1], rd[:],
                                    mybir.AluOpType.mult)
            w = w_pool.tile([TT, K], BF16)
            nc.vector.tensor_scalar_mul(w[:], pu[:], scl[:])
            # acc[k, 0:D] += w^T x ; acc[k, D] += w^T 1
            nc.tensor.matmul(acc[:], w[:], xbf[:],
                             start=(ti == 0), stop=(ti == ntiles - 1))
        # epilogue: e = acc[:, :D]/acc[:, D] - mu
        rn = sc_pool.tile([K, 1], F32, tag="rn")
        nc.vector.reciprocal(rn[:], acc[:, D:D + 1])
        ex = res_pool.tile([K, D], F32, tag="ex")
        nc.vector.tensor_scalar_mul(ex[:], acc[:, 0:D], rn[:])
        res = res_pool.tile([K, D], F32, tag="res")
        nc.vector.tensor_sub(res[:], ex[:], mu_sb[:])
        nc.sync.dma_start(out_d[b], res[:])


def make_inputs(x, weights, mu, s, bpc=BPC, t=T):
    """Host-side prep: shard + precompute small replicated tensors."""
    ntiles = t // TT
    s = np.asarray(s, dtype=np.float32)
    s0 = float(s[0])
    if not np.allclose(s, s0):
        raise NotImplementedError("kernel assumes uniform s (as in setup)")
    mu = np.ascontiguousarray(mu, dtype=np.float32)
    mu2t = (2.0 * s0 * mu).T.astype(ml_dtypes.bfloat16)      # [D, K]
    muT2 = np.concatenate([mu2t[:128], mu2t[128:]], axis=1)  # [128, 2K]
    c = s0 * np.sum(mu.astype(np.float64) ** 2, axis=1)
    u = np.exp(-c).astype(ml_dtypes.bfloat16)                # [K]
    urep = np.broadcast_to(u, (128, K)).copy()
    ident = np.eye(128, dtype=ml_dtypes.bfloat16)
    ncores = x.shape[0] // bpc
    in_maps = []
    for ci in range(ncores):
        xs = np.ascontiguousarray(x[ci * bpc:(ci + 1) * bpc, :t],
                                  dtype=np.float32)
        wsl = weights[ci * bpc:(ci + 1) * bpc, :t].astype(np.float32)
        wsT = np.ascontiguousarray(
            wsl.reshape(bpc, ntiles, TT).transpose(0, 2, 1))  # [bpc,128,nt]
        in_maps.append({
            "x": xs, "wsT": wsT, "muT2": muT2, "urep": urep,
            "mu": mu, "ident": ident,
        })
    return in_maps


_CACHE = {}


def _get_program():
    if "nc" not in _CACHE:
        _CACHE["nc"] = build_program()
    return _CACHE["nc"]


def kernel(x, weights, mu, s):
    x = np.asarray(x)
    weights = np.asarray(weights)
    mu = np.asarray(mu, dtype=np.float32)
    s = np.asarray(s, dtype=np.float32)
    nc = _get_program()
    in_maps = make_inputs(x, weights, mu, s)
    res = run_bass_kernel_spmd(nc, in_maps, core_ids=list(range(NCORES)))
    outs = [res.results[ci]["out"].reshape(BPC, K * D)
            for ci in range(NCORES)]
    return np.concatenate(outs, axis=0).astype(np.float32)


if __name__ == "__main__":
    rng = np.random.default_rng(0)
    x = rng.standard_normal((B, T, D), dtype=np.float32)
    w = rng.random((B, T), dtype=np.float32)
    mu = (0.1 * rng.standard_normal((K, D))).astype(np.float32)
    s = np.ones((K,), dtype=np.float32)
    out = kernel(x, weights=w, mu=mu, s=s)
    print("out", out.shape, out.dtype)



# revision 2
# speedup vs baseline: 44.2030x; 44.2030x over previous
"""Trainium2 Bass kernel for the LDE1D vq_codebook problem, v7.

v6 + instruction grouping: per-op fixed costs (~100-180ns on ACT/DVE)
dominated v6's schedule, so the softmax chain now processes G=4 token
tiles per instruction. The u[k] multiply is folded into the q matmul
group as a +ln(u[k]) bias row (PE rank-1 matmul, last in the PSUM
accumulation group — bank pending-zero semantics make this exact).
accT0/accT1/nacc share one PSUM bank (single accumulation group).
PSUM->SBUF copy of x^T is split DVE/ACT by column range.

Stages per 4-tile group (offsets in groups): A+0 PE 8 transposes |
B+1 DVE+ACT copy | C+2 PE 8 q-matmuls + lnu row | D+3 ACT exp |
E+4 DVE reduce/recip/scl | F+5 Pool w-scale | G+6 PE 12 acc matmuls.

Math identical to v4 (see kernel_v4 docstring); host epilogue
e[k,d] = accT[d,k]/nacc[k] - mu[k,d].
"""

import sys
from contextlib import ExitStack

import numpy as np

sys.path.insert(0, "/opt/trn_rl_repo")

import ml_dtypes

import concourse.bass as bass
import concourse.tile as tile
from concourse import bacc, mybir
from concourse.bass_utils import run_bass_kernel_spmd

BF16 = mybir.dt.bfloat16
F32 = mybir.dt.float32

B, T, D, K = 64, 4096, 256, 64
NCORES = 8
BPC = B // NCORES
TT = 128
G = 4                       # tiles per group
CSPLIT = 160                # copy column split: DVE [0:CSPLIT], ACT rest

OFF_A, OFF_B, OFF_C, OFF_D, OFF_E, OFF_F, OFF_G = 0, 1, 2, 3, 4, 5, 6
DRAIN = OFF_G + 1


def build_program(bpc=BPC, t=T, reps=1, trn_type="TRN2"):
    ntiles = t // TT
    assert ntiles % G == 0
    nc = bacc.Bacc(trn_type, target_bir_lowering=False, debug=False,
                   num_devices=NCORES)
    x_d = nc.dram_tensor("x", [bpc, TT, ntiles, D], BF16,
                         kind="ExternalInput").ap()
    wsT_d = nc.dram_tensor("wsT", [TT, bpc * ntiles], F32,
                           kind="ExternalInput").ap()
    muT2_d = nc.dram_tensor("muT2", [128, 2 * K], BF16,
                            kind="ExternalInput").ap()
    lnu_d = nc.dram_tensor("lnu", [1, G * K], BF16, kind="ExternalInput").ap()
    ident_d = nc.dram_tensor("ident", [128, 128], BF16,
                             kind="ExternalInput").ap()
    accT_d = nc.dram_tensor("accT", [bpc, 128, 2, K], F32,
                            kind="ExternalOutput").ap()
    nacc_d = nc.dram_tensor("nacc", [1, bpc * K], F32,
                            kind="ExternalOutput").ap()

    with tile.TileContext(nc) as tc, ExitStack() as ctx:
        _body(ctx, tc, accT_d, nacc_d, x_d, wsT_d, muT2_d, lnu_d, ident_d,
              bpc, ntiles, reps)
    nc.compile()
    return nc


def _body(ctx, tc, accT_d, nacc_d, x_d, wsT_d, muT2_d, lnu_d, ident_d,
          bpc, ntiles, reps):
    nc = tc.nc
    ngroups = ntiles // G
    const = ctx.enter_context(tc.tile_pool(name="const", bufs=1))
    muT2 = const.tile([128, 2 * K], BF16)
    nc.sync.dma_start(muT2[:], muT2_d[:])
    lnu = const.tile([1, G * K], BF16)
    nc.sync.dma_start(lnu[:], lnu_d[:])
    ident = const.tile([128, 128], BF16)
    nc.sync.dma_start(ident[:], ident_d[:])
    ones = const.tile([TT, K], BF16)
    nc.gpsimd.memset(ones[:], 1.0)
    ones1 = const.tile([1, TT], BF16)
    nc.gpsimd.memset(ones1[:], 1.0)
    wsall = const.tile([TT, bpc * ntiles], F32)
    nc.sync.dma_start(wsall[:], wsT_d[:])
    naccs = const.tile([1, bpc * K], F32)

    xb_pool = ctx.enter_context(tc.tile_pool(name="xb", bufs=2))
    xt_pool = ctx.enter_context(tc.tile_pool(name="xt", bufs=3))
    p_pool = ctx.enter_context(tc.tile_pool(name="p", bufs=3))
    w_pool = ctx.enter_context(tc.tile_pool(name="w", bufs=3))
    dt_pool = ctx.enter_context(tc.tile_pool(name="dt", bufs=3))
    rd_pool = ctx.enter_context(tc.tile_pool(name="rd", bufs=3))
    scl_pool = ctx.enter_context(tc.tile_pool(name="scl", bufs=3))
    res_pool = ctx.enter_context(tc.tile_pool(name="res", bufs=2))
    pt_psum = ctx.enter_context(tc.tile_pool(name="pt", bufs=3, space="PSUM"))
    pq_psum = ctx.enter_context(tc.tile_pool(name="pq", bufs=3, space="PSUM"))
    pa_psum = ctx.enter_context(tc.tile_pool(name="pa", bufs=2, space="PSUM"))

    nbat = reps * bpc
    ntotg = nbat * ngroups
    xb = {}
    pt_t, xt_t, pq_t, p_t, w_t, rd_t, scl_t = {}, {}, {}, {}, {}, {}, {}
    accb = {}
    next_nb = 0

    for it in range(ntotg + DRAIN):
        while next_nb < nbat and next_nb * ngroups <= it + 2:
            xbt = xb_pool.tile([TT, ntiles, D], BF16)
            nc.sync.dma_start(xbt[:], x_d[next_nb % bpc])
            xb[next_nb] = xbt
            next_nb += 1

        gg = it - OFF_A
        if 0 <= gg < ntotg:  # A: PE transposes (8 per group)
            nb, g = gg // ngroups, gg % ngroups
            pt = pt_psum.tile([128, G, 256], BF16)
            for j in range(G):
                xin = xb[nb][:, g * G + j, :]
                nc.tensor.transpose(pt[:, j, 0:128], xin[:, 0:128], ident[:])
                nc.tensor.transpose(pt[:, j, 128:256], xin[:, 128:256],
                                    ident[:])
            pt_t[gg] = pt

        gg = it - OFF_B
        if 0 <= gg < ntotg:  # B: copy PSUM->SBUF split DVE/ACT
            pt = pt_t.pop(gg)
            xt = xt_pool.tile([128, G, 256], BF16)
            nc.vector.tensor_copy(xt[:, :, 0:CSPLIT], pt[:, :, 0:CSPLIT])
            nc.scalar.copy(xt[:, :, CSPLIT:256], pt[:, :, CSPLIT:256])
            xt_t[gg] = xt

        gg = it - OFF_C
        if 0 <= gg < ntotg:  # C: PE q matmuls, one PSUM group + lnu row
            xt = xt_t.pop(gg)
            pq = pq_psum.tile([TT, G, K], F32)
            for j in range(G):
                nc.tensor.matmul(pq[:, j, :], xt[:, j, 0:128], muT2[:, 0:K],
                                 start=(j == 0), stop=False)
                nc.tensor.matmul(pq[:, j, :], xt[:, j, 128:256],
                                 muT2[:, K:2 * K], start=False, stop=False)
            nc.tensor.matmul(pq[:], ones1[:], lnu[:],
                             start=False, stop=True)
            pq_t[gg] = pq

        gg = it - OFF_D
        if 0 <= gg < ntotg:  # D: ACT exp (includes u via lnu bias row)
            p = p_pool.tile([TT, G, K], BF16)
            nc.scalar.activation(p[:], pq_t.pop(gg)[:],
                                 mybir.ActivationFunctionType.Exp)
            p_t[gg] = p

        gg = it - OFF_E
        if 0 <= gg < ntotg:  # E: DVE dt, rd, scl
            nb, g = gg // ngroups, gg % ngroups
            p = p_t[gg]
            dt = dt_pool.tile([TT, G], F32)
            nc.vector.tensor_reduce(dt[:], p[:], mybir.AxisListType.X,
                                    mybir.AluOpType.add)
            rd = rd_pool.tile([TT, G], F32)
            nc.vector.reciprocal(rd[:], dt[:])
            scl = scl_pool.tile([TT, G, 1], F32)
            col = (nb % bpc) * ntiles + g * G
            nc.vector.tensor_tensor(
                scl[:, :, 0], wsall[:, col:col + G], rd[:],
                mybir.AluOpType.mult)
            scl_t[gg] = scl

        gg = it - OFF_F
        if 0 <= gg < ntotg:  # F: Pool w = p * scl (broadcast over k)
            p = p_t.pop(gg)
            scl = scl_t.pop(gg)
            w = w_pool.tile([TT, G, K], BF16)
            sb, wb = bass.broadcast_tensor_aps(scl[:], w[:])
            nc.gpsimd.tensor_tensor(w[:], p[:], sb, mybir.AluOpType.mult)
            w_t[gg] = w

        gg = it - OFF_G
        if 0 <= gg < ntotg:  # G: PE acc matmuls (+ batch epilogue)
            nb, g = gg // ngroups, gg % ngroups
            if g == 0:
                accb[nb] = pa_psum.tile([128, 3 * K], F32, name="accb")
            ab = accb[nb]
            w = w_t.pop(gg)
            for j in range(G):
                ti = g * G + j
                first = ti == 0
                last = ti == ntiles - 1
                xin = xb[nb][:, ti, :]
                wj = w[:, j, :]
                nc.tensor.matmul(ab[:, 0:K], xin[:, 0:128], wj,
                                 start=first, stop=last,
                                 skip_group_check=True)
                nc.tensor.matmul(ab[:, K:2 * K], xin[:, 128:256], wj,
                                 start=False, stop=last,
                                 skip_group_check=True)
                nc.tensor.matmul(ab[0:K, 2 * K:3 * K], ones[:, 0:K], wj,
                                 start=False, stop=last,
                                 skip_group_check=True)
            if g == ngroups - 1:
                b = nb % bpc
                accs = res_pool.tile([128, 2, K], F32, tag="accs")
                nc.scalar.copy(accs[:, 0, :], ab[:, 0:K])
                nc.scalar.copy(accs[:, 1, :], ab[:, K:2 * K])
                nc.scalar.copy(naccs[:, b * K:(b + 1) * K],
                               ab[0:1, 2 * K:3 * K])
                nc.sync.dma_start(accT_d[b], accs[:])
                del accb[nb], xb[nb]
    nc.sync.dma_start(nacc_d[:], naccs[:])


def make_inputs(x, weights, mu, s, bpc=BPC, t=T):
    """Host-side prep: shard + precompute small replicated tensors."""
    ntiles = t // TT
    s = np.asarray(s, dtype=np.float32)
    s0 = float(s[0])
    if not np.allclose(s, s0):
        raise NotImplementedError("kernel assumes uniform s (as in setup)")
    mu = np.ascontiguousarray(mu, dtype=np.float32)
    mu2t = (2.0 * s0 * mu).T.astype(ml_dtypes.bfloat16)      # [D, K]
    muT2 = np.concatenate([mu2t[:128], mu2t[128:]], axis=1)  # [128, 2K]
    c = s0 * np.sum(mu.astype(np.float64) ** 2, axis=1)
    lnu = np.tile(-c.astype(np.float32), G).reshape(1, G * K)
    lnu = lnu.astype(ml_dtypes.bfloat16)
    ident = np.eye(128, dtype=ml_dtypes.bfloat16)
    ncores = x.shape[0] // bpc
    xbf = np.asarray(x[:, :t], dtype=ml_dtypes.bfloat16)
    xbf = xbf.reshape(x.shape[0], ntiles, TT, D).transpose(0, 2, 1, 3)
    ws = np.asarray(weights[:, :t], dtype=np.float32)
    wsT = ws.reshape(x.shape[0], ntiles, TT).transpose(2, 0, 1)
    in_maps = []
    for ci in range(ncores):
        sl = slice(ci * bpc, (ci + 1) * bpc)
        in_maps.append({
            "x": np.ascontiguousarray(xbf[sl]),
            "wsT": np.ascontiguousarray(
                wsT[:, sl].reshape(TT, bpc * ntiles)),
            "muT2": muT2, "lnu": lnu, "ident": ident,
        })
    return in_maps


OUTPUT_NAMES = ["accT", "nacc"]


def postprocess(outs, mu, bpc=BPC):
    accT = outs["accT"]                         # [bpc, 128, 2, K]
    nacc = outs["nacc"].reshape(bpc, K)
    A = accT.transpose(0, 3, 2, 1).reshape(bpc, K, D)   # [b, k, d]
    e = A / nacc.reshape(bpc, K, 1) - mu[None]
    return e.reshape(bpc, K * D).astype(np.float32)


_CACHE = {}


def _get_program():
    if "nc" not in _CACHE:
        _CACHE["nc"] = build_program()
    return _CACHE["nc"]


def kernel(x, weights, mu, s):
    x = np.asarray(x)
    weights = np.asarray(weights)
    mu = np.asarray(mu, dtype=np.float32)
    s = np.asarray(s, dtype=np.float32)
    nc = _get_program()
    in_maps = make_inputs(x, weights, mu, s)
    res = run_bass_kernel_spmd(nc, in_maps, core_ids=list(range(NCORES)))
    outs = [postprocess(res.results[ci], mu) for ci in range(NCORES)]
    return np.concatenate(outs, axis=0)


if __name__ == "__main__":
    rng = np.random.default_rng(0)
    x = rng.standard_normal((B, T, D), dtype=np.float32)
    w = rng.random((B, T), dtype=np.float32)
    mu = (0.1 * rng.standard_normal((K, D))).astype(np.float32)
    s = np.ones((K,), dtype=np.float32)
    out = kernel(x, weights=w, mu=mu, s=s)
    print("out", out.shape, out.dtype)


# revision 4
# speedup vs baseline: 46.4792x; 1.0515x over previous
"""Trainium2 Bass kernel for the LDE1D vq_codebook problem, v7.

v6 + instruction grouping: per-op fixed costs (~100-180ns on ACT/DVE)
dominated v6's schedule, so the softmax chain now processes G=4 token
tiles per instruction. The u[k] multiply is folded into the q matmul
group as a +ln(u[k]) bias row (PE rank-1 matmul, last in the PSUM
accumulation group — bank pending-zero semantics make this exact).
accT0/accT1/nacc share one PSUM bank (single accumulation group).
PSUM->SBUF copy of x^T is split DVE/ACT by column range.

Stages per 4-tile group (offsets in groups): A+0 PE 8 transposes |
B+1 DVE+ACT copy | C+2 PE 8 q-matmuls + lnu row | D+3 ACT exp |
E+4 DVE reduce/recip/scl | F+5 Pool w-scale | G+6 PE 12 acc matmuls.

Math identical to v4 (see kernel_v4 docstring); host epilogue
e[k,d] = accT[d,k]/nacc[k] - mu[k,d].
"""

import sys
from contextlib import ExitStack

import numpy as np

sys.path.insert(0, "/opt/trn_rl_repo")

import ml_dtypes

import concourse.bass as bass
import concourse.tile as tile
from concourse import bacc, mybir
from concourse.bass_utils import run_bass_kernel_spmd

BF16 = mybir.dt.bfloat16
F32 = mybir.dt.float32

B, T, D, K = 64, 4096, 256, 64
NCORES = 8
BPC = B // NCORES
TT = 128
G = 4                       # tiles per group
CSPLIT = 176                # copy column split: DVE [0:CSPLIT], ACT rest

OFF_A, OFF_B, OFF_C, OFF_D, OFF_E, OFF_F, OFF_G = 0, 1, 2, 3, 4, 5, 6
DRAIN = OFF_G + 1


def build_program(bpc=BPC, t=T, reps=1, trn_type="TRN2"):
    ntiles = t // TT
    assert ntiles % G == 0
    nc = bacc.Bacc(trn_type, target_bir_lowering=False, debug=False,
                   num_devices=NCORES)
    x_d = nc.dram_tensor("x", [bpc, TT, ntiles, D], BF16,
                         kind="ExternalInput").ap()
    wsT_d = nc.dram_tensor("wsT", [TT, bpc * ntiles], F32,
                           kind="ExternalInput").ap()
    muT2_d = nc.dram_tensor("muT2", [128, 2 * K], BF16,
                            kind="ExternalInput").ap()
    lnu_d = nc.dram_tensor("lnu", [1, G * K], BF16, kind="ExternalInput").ap()
    ident_d = nc.dram_tensor("ident", [128, 128], BF16,
                             kind="ExternalInput").ap()
    accT_d = nc.dram_tensor("accT", [bpc, 128, 2, K], F32,
                            kind="ExternalOutput").ap()
    nacc_d = nc.dram_tensor("nacc", [1, bpc * K], F32,
                            kind="ExternalOutput").ap()

    with tile.TileContext(nc) as tc, ExitStack() as ctx:
        _body(ctx, tc, accT_d, nacc_d, x_d, wsT_d, muT2_d, lnu_d, ident_d,
              bpc, ntiles, reps)
    nc.compile()
    return nc


def _body(ctx, tc, accT_d, nacc_d, x_d, wsT_d, muT2_d, lnu_d, ident_d,
          bpc, ntiles, reps):
    nc = tc.nc
    ngroups = ntiles // G
    const = ctx.enter_context(tc.tile_pool(name="const", bufs=1))
    muT2 = const.tile([128, 2 * K], BF16)
    nc.sync.dma_start(muT2[:], muT2_d[:])
    lnu = const.tile([1, G * K], BF16)
    nc.sync.dma_start(lnu[:], lnu_d[:])
    ident = const.tile([128, 128], BF16)
    nc.sync.dma_start(ident[:], ident_d[:])
    ones = const.tile([TT, K], BF16)
    nc.gpsimd.memset(ones[:], 1.0)
    ones1 = const.tile([1, TT], BF16)
    nc.gpsimd.memset(ones1[:], 1.0)
    wsall = const.tile([TT, bpc * ntiles], F32)
    nc.sync.dma_start(wsall[:], wsT_d[:])
    naccs = const.tile([1, bpc * K], F32)

    xb_pool = ctx.enter_context(tc.tile_pool(name="xb", bufs=3))
    xt_pool = ctx.enter_context(tc.tile_pool(name="xt", bufs=3))
    p_pool = ctx.enter_context(tc.tile_pool(name="p", bufs=3))
    w_pool = ctx.enter_context(tc.tile_pool(name="w", bufs=3))
    dt_pool = ctx.enter_context(tc.tile_pool(name="dt", bufs=3))
    rd_pool = ctx.enter_context(tc.tile_pool(name="rd", bufs=3))
    scl_pool = ctx.enter_context(tc.tile_pool(name="scl", bufs=3))
    res_pool = ctx.enter_context(tc.tile_pool(name="res", bufs=2))
    pt_psum = ctx.enter_context(tc.tile_pool(name="pt", bufs=3, space="PSUM"))
    pq_psum = ctx.enter_context(tc.tile_pool(name="pq", bufs=3, space="PSUM"))
    pa_psum = ctx.enter_context(tc.tile_pool(name="pa", bufs=2, space="PSUM"))

    nbat = reps * bpc
    ntotg = nbat * ngroups
    xb = {}
    pt_t, xt_t, pq_t, p_t, w_t, rd_t, scl_t = {}, {}, {}, {}, {}, {}, {}
    accb = {}
    next_nb = 0

    for it in range(ntotg + DRAIN):
        while next_nb < nbat and next_nb * ngroups <= it + 10:
            xbt = xb_pool.tile([TT, ntiles, D], BF16)
            nsplit = min(8, ntiles)
            q4 = ntiles // nsplit
            for hh in range(nsplit):
                nc.sync.dma_start(
                    xbt[:, hh * q4:(hh + 1) * q4, :],
                    x_d[next_nb % bpc][:, hh * q4:(hh + 1) * q4, :])
            xb[next_nb] = xbt
            next_nb += 1

        gg = it - OFF_A
        if 0 <= gg < ntotg:  # A: PE transposes (8 per group)
            nb, g = gg // ngroups, gg % ngroups
            pt = pt_psum.tile([128, G, 256], BF16)
            for j in range(G):
                xin = xb[nb][:, g * G + j, :]
                nc.tensor.transpose(pt[:, j, 0:128], xin[:, 0:128], ident[:])
                nc.tensor.transpose(pt[:, j, 128:256], xin[:, 128:256],
                                    ident[:])
            pt_t[gg] = pt

        gg = it - OFF_B
        if 0 <= gg < ntotg:  # B: copy PSUM->SBUF split DVE/ACT
            pt = pt_t.pop(gg)
            xt = xt_pool.tile([128, G, 256], BF16)
            nc.vector.tensor_copy(xt[:, :, 0:CSPLIT], pt[:, :, 0:CSPLIT])
            nc.scalar.copy(xt[:, :, CSPLIT:256], pt[:, :, CSPLIT:256])
            xt_t[gg] = xt

        gg = it - OFF_C
        if 0 <= gg < ntotg:  # C: PE q matmuls, one PSUM group + lnu row
            xt = xt_t.pop(gg)
            pq = pq_psum.tile([TT, G, K], F32)
            for j in range(G):
                nc.tensor.matmul(pq[:, j, :], xt[:, j, 0:128], muT2[:, 0:K],
                                 start=(j == 0), stop=False)
                nc.tensor.matmul(pq[:, j, :], xt[:, j, 128:256],
                                 muT2[:, K:2 * K], start=False, stop=False)
            nc.tensor.matmul(pq[:], ones1[:], lnu[:],
                             start=False, stop=True)
            pq_t[gg] = pq

        gg = it - OFF_D
        if 0 <= gg < ntotg:  # D: ACT exp (includes u via lnu bias row)
            p = p_pool.tile([TT, G, K], BF16)
            nc.scalar.activation(p[:], pq_t.pop(gg)[:],
                                 mybir.ActivationFunctionType.Exp)
            p_t[gg] = p

        gg = it - OFF_E
        if 0 <= gg < ntotg:  # E: DVE dt, rd, scl
            nb, g = gg // ngroups, gg % ngroups
            p = p_t[gg]
            dt = dt_pool.tile([TT, G], F32)
            nc.vector.tensor_reduce(dt[:], p[:], mybir.AxisListType.X,
                                    mybir.AluOpType.add)
            rd = rd_pool.tile([TT, G], F32)
            nc.vector.reciprocal(rd[:], dt[:])
            scl = scl_pool.tile([TT, G, 1], F32)
            col = (nb % bpc) * ntiles + g * G
            nc.vector.tensor_tensor(
                scl[:, :, 0], wsall[:, col:col + G], rd[:],
                mybir.AluOpType.mult)
            scl_t[gg] = scl

        gg = it - OFF_F
        if 0 <= gg < ntotg:  # F: Pool w = p * scl (broadcast over k)
            p = p_t.pop(gg)
            scl = scl_t.pop(gg)
            w = w_pool.tile([TT, G, K], BF16)
            sb, wb = bass.broadcast_tensor_aps(scl[:], w[:])
            nc.gpsimd.tensor_tensor(w[:], p[:], sb, mybir.AluOpType.mult)
            w_t[gg] = w

        gg = it - OFF_G
        if 0 <= gg < ntotg:  # G: PE acc matmuls (+ batch epilogue)
            nb, g = gg // ngroups, gg % ngroups
            if g == 0:
                accb[nb] = pa_psum.tile([128, 3 * K], F32, name="accb")
            ab = accb[nb]
            w = w_t.pop(gg)
            for j in range(G):
                ti = g * G + j
                first = ti == 0
                last = ti == ntiles - 1
                xin = xb[nb][:, ti, :]
                wj = w[:, j, :]
                nc.tensor.matmul(ab[:, 0:K], xin[:, 0:128], wj,
                                 start=first, stop=last,
                                 skip_group_check=True)
                nc.tensor.matmul(ab[:, K:2 * K], xin[:, 128:256], wj,
                                 start=False, stop=last,
                                 skip_group_check=True)
                nc.tensor.matmul(ab[0:K, 2 * K:3 * K], ones[:, 0:K], wj,
                                 start=False, stop=last,
                                 skip_group_check=True)
            if g == ngroups - 1:
                b = nb % bpc
                accs = res_pool.tile([128, 2, K], F32, tag="accs")
                nc.vector.tensor_copy(accs[:, 0, :], ab[:, 0:K])
                nc.scalar.copy(accs[:, 1, :], ab[:, K:2 * K])
                nc.scalar.copy(naccs[:, b * K:(b + 1) * K],
                               ab[0:1, 2 * K:3 * K])
                nc.sync.dma_start(accT_d[b], accs[:])
                del accb[nb], xb[nb]
    nc.sync.dma_start(nacc_d[:], naccs[:])


def make_inputs(x, weights, mu, s, bpc=BPC, t=T):
    """Host-side prep: shard + precompute small replicated tensors."""
    ntiles = t // TT
    s = np.asarray(s, dtype=np.float32)
    s0 = float(s[0])
    if not np.allclose(s, s0):
        raise NotImplementedError("kernel assumes uniform s (as in setup)")
    mu = np.ascontiguousarray(mu, dtype=np.float32)
    mu2t = (2.0 * s0 * mu).T.astype(ml_dtypes.bfloat16)      # [D, K]
    muT2 = np.concatenate([mu2t[:128], mu2t[128:]], axis=1)  # [128, 2K]
    c = s0 * np.sum(mu.astype(np.float64) ** 2, axis=1)
    lnu = np.tile(-c.astype(np.float32), G).reshape(1, G * K)
    lnu = lnu.astype(ml_dtypes.bfloat16)
    ident = np.eye(128, dtype=ml_dtypes.bfloat16)
    ncores = x.shape[0] // bpc
    xbf = np.asarray(x[:, :t], dtype=ml_dtypes.bfloat16)
    xbf = xbf.reshape(x.shape[0], ntiles, TT, D).transpose(0, 2, 1, 3)
    ws = np.asarray(weights[:, :t], dtype=np.float32)
    wsT = ws.reshape(x.shape[0], ntiles, TT).transpose(2, 0, 1)
    in_maps = []
    for ci in range(ncores):
        sl = slice(ci * bpc, (ci + 1) * bpc)
        in_maps.append({
            "x": np.ascontiguousarray(xbf[sl]),
            "wsT": np.ascontiguousarray(
                wsT[:, sl].reshape(TT, bpc * ntiles)),
            "muT2": muT2, "lnu": lnu, "ident": ident,
        })
    return in_maps


OUTPUT_NAMES = ["accT", "nacc"]


def postprocess(outs, mu, bpc=BPC):
    accT = outs["accT"]                         # [bpc, 128, 2, K]
    nacc = outs["nacc"].reshape(bpc, K)
    A = accT.transpose(0, 3, 2, 1).reshape(bpc, K, D)   # [b, k, d]
    e = A / nacc.reshape(bpc, K, 1) - mu[None]
    return e.reshape(bpc, K * D).astype(np.float32)


_CACHE = {}


def _get_program():
    if "nc" not in _CACHE:
        _CACHE["nc"] = build_program()
    return _CACHE["nc"]


def kernel(x, weights, mu, s):
    x = np.asarray(x)
    weights = np.asarray(weights)
    mu = np.asarray(mu, dtype=np.float32)
    s = np.asarray(s, dtype=np.float32)
    nc = _get_program()
    in_maps = make_inputs(x, weights, mu, s)
    res = run_bass_kernel_spmd(nc, in_maps, core_ids=list(range(NCORES)))
    outs = [postprocess(res.results[ci], mu) for ci in range(NCORES)]
    return np.concatenate(outs, axis=0)


if __name__ == "__main__":
    rng = np.random.default_rng(0)
    x = rng.standard_normal((B, T, D), dtype=np.float32)
    w = rng.random((B, T), dtype=np.float32)
    mu = (0.1 * rng.standard_normal((K, D))).astype(np.float32)
    s = np.ones((K,), dtype=np.float32)
    out = kernel(x, weights=w, mu=mu, s=s)
    print("out", out.shape, out.dtype)
